# revision 18
# baseline (speedup 1.0000x reference)
"""Distributed Trainium2 kernel for AdaptiveLowRank (softmax-weighted sum of
16 linear maps + LayerNorm), SPMD across 8 NeuronCores.

Strategy: data-parallel over tokens; the 16 weight matrices are sharded over
both r and output-dim so each core reads only 1/8 of Ws. The o-slice of
W_eff = sum_r softmax(rank_weights)_r * W_r is combined on the TensorEngine
(p_r*I diagonal matmuls accumulating in PSUM f32), transposed on the PE,
and AllGathered in two d-chunks so the gather overlaps the start of the
main matmul. LayerNorm stats run on DVE (bn_stats); the normalize runs on
ACT straight out of PSUM with the mask folded into the per-token affine.

DMA engine assignment (each engine queue is in-order, so streams that must
not block each other live on different engines):
  SP (sync):   ws loads, strided W_eff^T moving-tile loads
  ACT (scalar): consts, x^T loads, cc_in bounce writes
  GPSIMD:      collectives, output stores
"""

import numpy as np
import ml_dtypes

import concourse.bass as bass
import concourse.mybir as mybir
import concourse.tile as tile
from concourse import bacc
from concourse import bass_utils

N_CORES = 8
B, S, D = 4, 2048, 1024
R = 16
T = (B * S) // N_CORES          # tokens per core
OL = D // N_CORES               # output rows owned per core
TB = T // 128                   # token blocks per core
DB = D // 128                   # contraction blocks
DH = D // 2                     # d-half size (AG chunk)
LN_EPS = 1e-5
MIN_RANK, MAX_RANK = 1, 16

BF16 = mybir.dt.bfloat16
F32 = mybir.dt.float32
NP_BF16 = ml_dtypes.bfloat16

_cached = {}


def _build(affine: bool):
    nc = bacc.Bacc("TRN2", target_bir_lowering=False, debug=False,
                   num_devices=N_CORES)

    xst = nc.dram_tensor("xst", [D, T], BF16, kind="ExternalInput")
    ws = nc.dram_tensor("ws", [R, OL, D], BF16, kind="ExternalInput")
    diag_in = nc.dram_tensor("diag", [128, R * 128], BF16,
                             kind="ExternalInput")
    ident_in = nc.dram_tensor("ident", [128, 128], BF16,
                              kind="ExternalInput")
    maskc_in = nc.dram_tensor("maskc", [128, TB], F32, kind="ExternalInput")
    if affine:
        gamma_in = nc.dram_tensor("gammab", [128, D], BF16,
                                  kind="ExternalInput")
        beta_in = nc.dram_tensor("betab", [128, D], BF16,
                                 kind="ExternalInput")
    out = nc.dram_tensor("out", [T, D], F32, kind="ExternalOutput")

    with tile.TileContext(nc) as tc:
        with (
            tc.tile_pool(name="dram", bufs=1, space="DRAM") as dram,
            tc.tile_pool(name="consts", bufs=1) as consts,
            tc.tile_pool(name="wld", bufs=16) as wld,
            tc.tile_pool(name="accsb", bufs=2) as accsb,
            tc.tile_pool(name="wtsb", bufs=4) as wtsb,
            tc.tile_pool(name="xt", bufs=1) as xtp,
            tc.tile_pool(name="wmov", bufs=1) as wmov,
            tc.tile_pool(name="psum_w", bufs=2, space="PSUM") as psum_w,
            tc.tile_pool(name="psum_y", bufs=4, space="PSUM") as psum_y,
            tc.tile_pool(name="ln", bufs=3) as lnp,
            tc.tile_pool(name="stats", bufs=3) as stats,
        ):
            # consts on ACT queue (tiny, first)
            diag = consts.tile([128, R * 128], BF16, name="diag")
            nc.scalar.dma_start(diag[:], diag_in[:])
            ident = consts.tile([128, 128], BF16, name="ident")
            nc.scalar.dma_start(ident[:], ident_in[:])
            maskc = consts.tile([128, TB], F32, name="maskc")
            nc.scalar.dma_start(maskc[:], maskc_in[:])
            if affine:
                gammab = consts.tile([128, D], BF16, name="gammab")
                nc.scalar.dma_start(gammab[:], gamma_in[:])
                betab = consts.tile([128, D], BF16, name="betab")
                nc.scalar.dma_start(betab[:], beta_in[:])

            # ACT table pre-warm (Identity + Sqrt)
            warm = stats.tile([128, 2], F32, tag="warm")
            nc.scalar.activation(warm[:, 0:1], maskc[:, 0:1],
                                 mybir.ActivationFunctionType.Identity,
                                 bias=0.0, scale=1.0)
            nc.scalar.sqrt(warm[:, 1:2], maskc[:, 0:1])

            # x^T tiles (plain loads, host pre-transposed) on ACT queue
            xt = []
            for k in range(DB):
                t_ = xtp.tile([128, T], BF16, tag=f"xt{k}")
                nc.scalar.dma_start(t_[:], xst[k * 128:(k + 1) * 128, :])
                xt.append(t_)

            # ws loads on SP queue (pipelined into the PE combine)
            wtiles = {}
            for h in range(2):
                for r in range(R):
                    w_r = wld.tile([OL, DH], BF16, tag="wld")
                    nc.sync.dma_start(
                        w_r[:], ws[r][:, h * DH:(h + 1) * DH])
                    wtiles[(h, r)] = w_r

            # ---- W-combine on PE -> PE transpose -> chunked AllGather ----
            cc_outs = []
            for h in range(2):
                pw = psum_w.tile([128, DH], F32, tag="pw")
                for r in range(R):
                    nc.tensor.matmul(pw[:], diag[:, r * 128:(r + 1) * 128],
                                     wtiles[(h, r)][:],
                                     start=(r == 0), stop=(r == R - 1))
                acc_sb = accsb.tile([128, DH], BF16, tag="accsb")
                nc.scalar.copy(acc_sb[:], pw[:])
                # transpose the half's 4 d-blocks on the PE
                cc_in = dram.tile([4 * 128, OL], BF16, name=f"cc_in{h}")
                for j in range(4):
                    pt = psum_w.tile([128, 128], BF16, tag="pt")
                    nc.tensor.transpose(
                        pt[:], acc_sb[:, j * 128:(j + 1) * 128], ident[:])
                    wt_sb = wtsb.tile([128, 128], BF16, tag="wtsb")
                    nc.scalar.copy(wt_sb[:], pt[:])
                    nc.scalar.dma_start(
                        cc_in[j * 128:(j + 1) * 128, :], wt_sb[:])
                cc_out = dram.tile([N_CORES * 4 * 128, OL], BF16,
                                   name=f"cc_out{h}", addr_space="Shared")
                nc.gpsimd.collective_compute(
                    "AllGather",
                    mybir.AluOpType.bypass,
                    replica_groups=[list(range(N_CORES))],
                    ins=[cc_in[:].opt()],
                    outs=[cc_out[:].opt()],
                )
                cc_outs.append(cc_out)

            # ---- W_eff^T moving tiles via strided loads on SP queue ----
            # cc_out rows: r*512 + j*128 + d', cols: o_local
            wm = []
            for k in range(DB):
                h, j = k // 4, k % 4
                src = cc_outs[h][:].rearrange(
                    "(r j p) f -> j p r f", r=N_CORES, j=4)[j]
                t_ = wmov.tile([128, D], BF16, tag=f"wm{k}")
                dst = t_[:].rearrange("p (r f) -> p r f", r=N_CORES)
                nc.sync.dma_start(dst, src)
                wm.append(t_)

            # ---- main matmuls + LayerNorm per token block ----
            for t in range(TB):
                ph0 = psum_y.tile([128, 512], F32, tag="py")
                ph1 = psum_y.tile([128, 512], F32, tag="py")
                ph = [ph0, ph1]
                for k in range(DB):
                    lhsT = xt[k][:, t * 128:(t + 1) * 128]
                    for h in range(2):
                        nc.tensor.matmul(ph[h][:], lhsT,
                                         wm[k][:, h * 512:(h + 1) * 512],
                                         start=(k == 0), stop=(k == DB - 1))

                # LN stats on DVE: bn_stats per half -> bn_aggr
                bs = stats.tile([128, 12], F32, tag="bs")
                for h in range(2):
                    nc.vector.bn_stats(bs[:, h * 6:(h + 1) * 6], ph[h][:])
                mv = stats.tile([128, 8], F32, tag="mv")
                nc.vector.bn_aggr(mv[:, 0:2], bs[:])
                # a_pre = m^2*var + eps; rs = 1/sqrt(a_pre)
                m = maskc[:, t:t + 1]
                nc.vector.tensor_scalar(
                    mv[:, 2:3], m, m, None, mybir.AluOpType.mult)
                nc.vector.tensor_scalar(
                    mv[:, 3:4], mv[:, 1:2], mv[:, 2:3], LN_EPS,
                    mybir.AluOpType.mult, mybir.AluOpType.add)
                nc.scalar.sqrt(mv[:, 4:5], mv[:, 3:4])
                nc.vector.reciprocal(mv[:, 5:6], mv[:, 4:5])
                # a = m*rs ; b = -mean*a
                nc.vector.tensor_scalar(
                    mv[:, 6:7], mv[:, 5:6], m, None, mybir.AluOpType.mult)
                nc.vector.tensor_scalar(
                    mv[:, 7:8], mv[:, 6:7], mv[:, 0:1], -1.0,
                    mybir.AluOpType.mult, mybir.AluOpType.mult)

                # normalize from PSUM on ACT: z = y*a + b
                if affine:
                    zsb = lnp.tile([128, D], BF16, tag="zsb")
                    for h in range(2):
                        nc.scalar.activation(
                            zsb[:, h * 512:(h + 1) * 512], ph[h][:],
                            mybir.ActivationFunctionType.Identity,
                            bias=mv[:, 7:8], scale=mv[:, 6:7])
                    zg = lnp.tile([128, D], BF16, tag="zg")
                    nc.vector.tensor_tensor(zg[:], zsb[:], gammab[:],
                                            mybir.AluOpType.mult)
                    zf = lnp.tile([128, D], F32, tag="zf")
                    nc.vector.tensor_tensor(zf[:], zg[:], betab[:],
                                            mybir.AluOpType.add)
                else:
                    zf = lnp.tile([128, D], F32, tag="zf")
                    for h in range(2):
                        nc.scalar.activation(
                            zf[:, h * 512:(h + 1) * 512], ph[h][:],
                            mybir.ActivationFunctionType.Identity,
                            bias=mv[:, 7:8], scale=mv[:, 6:7])
                nc.gpsimd.dma_start(out[t * 128:(t + 1) * 128, :], zf[:])

    nc.compile()
    return nc


def _get_nc(affine: bool):
    key = f"nc_{affine}"
    if key not in _cached:
        _cached[key] = _build(affine)
    return _cached[key]


def _host_prep(x, mask, rank_weights, Ws, ln_gamma, ln_beta):
    rw = rank_weights.astype(np.float64)
    e = np.exp(rw - rw.max())
    probs64 = e / e.sum()
    ranks = np.arange(MIN_RANK, MAX_RANK + 1, dtype=np.float64)
    expected_rank = np.float32((probs64 * ranks).sum())
    rank_entropy = np.float32(-(probs64 * np.log(probs64 + 1e-8)).sum())
    probs = probs64.astype(np.float32)

    affine = not (np.all(ln_gamma == 1.0) and np.all(ln_beta == 0.0))

    x2 = np.ascontiguousarray(x.reshape(B * S, D))
    mask2 = mask.reshape(B * S)
    diag = np.zeros((128, R * 128), dtype=np.float32)
    for r in range(R):
        diag[np.arange(128), r * 128 + np.arange(128)] = probs[r]
    diag = diag.astype(NP_BF16)
    ident = np.eye(128, dtype=NP_BF16)
    if affine:
        gamma_b = np.ascontiguousarray(
            np.broadcast_to(ln_gamma[None, :], (128, D))).astype(NP_BF16)
        beta_b = np.ascontiguousarray(
            np.broadcast_to(ln_beta[None, :], (128, D))).astype(NP_BF16)

    in_maps = []
    for i in range(N_CORES):
        m = {
            "xst": np.ascontiguousarray(
                x2[i * T:(i + 1) * T].T).astype(NP_BF16),
            "ws": np.ascontiguousarray(
                Ws[:, i * OL:(i + 1) * OL, :]).astype(NP_BF16),
            "diag": diag,
            "ident": ident,
            "maskc": np.ascontiguousarray(
                mask2[i * T:(i + 1) * T].reshape(TB, 128).T
            ).astype(np.float32),
        }
        if affine:
            m["gammab"] = gamma_b
            m["betab"] = beta_b
        in_maps.append(m)
    return in_maps, affine, expected_rank, rank_entropy


def kernel(x, mask, rank_weights, Ws, ln_gamma, ln_beta):
    in_maps, affine, expected_rank, rank_entropy = _host_prep(
        x, mask, rank_weights, Ws, ln_gamma, ln_beta)
    nc = _get_nc(affine)
    _cached["in_maps"] = in_maps
    _cached["affine"] = affine
    res = bass_utils.run_bass_kernel_spmd(
        nc, in_maps, core_ids=list(range(N_CORES)))
    out = np.concatenate([res.results[i]["out"] for i in range(N_CORES)],
                         axis=0).reshape(B, S, D).astype(np.float32)
    return out, expected_rank, rank_entropy


# revision 21
# speedup vs baseline: 1.1176x; 1.1176x over previous
"""Distributed Trainium2 kernel for AdaptiveLowRank (softmax-weighted sum of
16 linear maps + LayerNorm), SPMD across 8 NeuronCores.

Strategy: data-parallel over tokens; the 16 weight matrices are sharded over
both r and output-dim so each core reads only 1/8 of Ws. The o-slice of
W_eff = sum_r softmax(rank_weights)_r * W_r is combined on the TensorEngine
(p_r*I diagonal matmuls accumulating in PSUM f32), transposed on the PE,
and AllGathered in two d-chunks so the gather overlaps the start of the
main matmul. LayerNorm stats run on DVE (bn_stats); the normalize runs on
ACT straight out of PSUM with the mask folded into the per-token affine.

DMA engine assignment (each engine queue is in-order, so streams that must
not block each other live on different engines):
  SP (sync):   ws loads, strided W_eff^T moving-tile loads
  ACT (scalar): consts, x^T loads, cc_in bounce writes
  GPSIMD:      collectives, output stores
"""

import numpy as np
import ml_dtypes

import concourse.bass as bass
import concourse.mybir as mybir
import concourse.tile as tile
from concourse import bacc
from concourse import bass_utils

N_CORES = 8
B, S, D = 4, 2048, 1024
R = 16
T = (B * S) // N_CORES          # tokens per core
OL = D // N_CORES               # output rows owned per core
TB = T // 128                   # token blocks per core
DB = D // 128                   # contraction blocks
DH = D // 2                     # d-half size (AG chunk)
LN_EPS = 1e-5
MIN_RANK, MAX_RANK = 1, 16

BF16 = mybir.dt.bfloat16
F32 = mybir.dt.float32
NP_BF16 = ml_dtypes.bfloat16

_cached = {}


def _build(affine: bool):
    nc = bacc.Bacc("TRN2", target_bir_lowering=False, debug=False,
                   num_devices=N_CORES)

    xst = nc.dram_tensor("xst", [D, T], BF16, kind="ExternalInput")
    ws = nc.dram_tensor("ws", [R, OL, D], BF16, kind="ExternalInput")
    diag_in = nc.dram_tensor("diag", [128, R * 128], BF16,
                             kind="ExternalInput")
    ident_in = nc.dram_tensor("ident", [128, 128], BF16,
                              kind="ExternalInput")
    maskc_in = nc.dram_tensor("maskc", [128, TB], F32, kind="ExternalInput")
    if affine:
        gamma_in = nc.dram_tensor("gammab", [128, D], BF16,
                                  kind="ExternalInput")
        beta_in = nc.dram_tensor("betab", [128, D], BF16,
                                 kind="ExternalInput")
    out = nc.dram_tensor("out", [T, D], F32, kind="ExternalOutput")

    with tile.TileContext(nc) as tc:
        with (
            tc.tile_pool(name="dram", bufs=1, space="DRAM") as dram,
            tc.tile_pool(name="consts", bufs=1) as consts,
            tc.tile_pool(name="wld", bufs=16) as wld,
            tc.tile_pool(name="accsb", bufs=2) as accsb,
            tc.tile_pool(name="wtsb", bufs=4) as wtsb,
            tc.tile_pool(name="xt", bufs=1) as xtp,
            tc.tile_pool(name="wmov", bufs=1) as wmov,
            tc.tile_pool(name="psum_w", bufs=2, space="PSUM") as psum_w,
            tc.tile_pool(name="psum_y", bufs=5, space="PSUM") as psum_y,
            tc.tile_pool(name="ln", bufs=3) as lnp,
            tc.tile_pool(name="stats", bufs=3) as stats,
        ):
            # consts on ACT queue (tiny, first)
            diag = consts.tile([128, R * 128], BF16, name="diag")
            nc.scalar.dma_start(diag[:], diag_in[:])
            ident = consts.tile([128, 128], BF16, name="ident")
            nc.scalar.dma_start(ident[:], ident_in[:])
            maskc = consts.tile([128, TB], F32, name="maskc")
            nc.scalar.dma_start(maskc[:], maskc_in[:])
            if affine:
                gammab = consts.tile([128, D], BF16, name="gammab")
                nc.scalar.dma_start(gammab[:], gamma_in[:])
                betab = consts.tile([128, D], BF16, name="betab")
                nc.scalar.dma_start(betab[:], beta_in[:])

            # ACT table pre-warm (Identity + Sqrt)
            warm = stats.tile([128, 2], F32, tag="warm")
            nc.scalar.activation(warm[:, 0:1], maskc[:, 0:1],
                                 mybir.ActivationFunctionType.Identity,
                                 bias=0.0, scale=1.0)
            nc.scalar.sqrt(warm[:, 1:2], maskc[:, 0:1])

            # ws loads on SP queue (pipelined into the PE combine)
            wtiles = {}
            for h in range(2):
                for r in range(R):
                    w_r = wld.tile([OL, DH], BF16, tag="wld")
                    nc.sync.dma_start(
                        w_r[:], ws[r][:, h * DH:(h + 1) * DH])
                    wtiles[(h, r)] = w_r

            # ---- W-combine on PE -> PE transpose -> one AllGather ----
            cc_in = dram.tile([D, OL], BF16, name="cc_in")
            for h in range(2):
                pw = psum_w.tile([128, DH], F32, tag="pw")
                for r in range(R):
                    nc.tensor.matmul(pw[:], diag[:, r * 128:(r + 1) * 128],
                                     wtiles[(h, r)][:],
                                     start=(r == 0), stop=(r == R - 1))
                acc_sb = accsb.tile([128, DH], BF16, tag="accsb")
                nc.scalar.copy(acc_sb[:], pw[:])
                # transpose the half's 4 d-blocks on the PE
                for j in range(4):
                    k = h * 4 + j
                    pt = psum_w.tile([128, 128], BF16, tag="pt", bufs=1)
                    nc.tensor.transpose(
                        pt[:], acc_sb[:, j * 128:(j + 1) * 128], ident[:])
                    wt_sb = wtsb.tile([128, 128], BF16, tag="wtsb")
                    nc.scalar.copy(wt_sb[:], pt[:])
                    nc.scalar.dma_start(
                        cc_in[k * 128:(k + 1) * 128, :], wt_sb[:])
            cc_out = dram.tile([N_CORES * D, OL], BF16,
                               name="cc_out", addr_space="Shared")
            nc.gpsimd.collective_compute(
                "AllGather",
                mybir.AluOpType.bypass,
                replica_groups=[list(range(N_CORES))],
                ins=[cc_in[:].opt()],
                outs=[cc_out[:].opt()],
            )

            # x^T tiles (host pre-transposed) on GPSIMD queue, right after
            # the collective doorbell: transfers fill the handshake window
            xt = []
            for k in range(DB):
                t_ = xtp.tile([128, T], BF16, tag=f"xt{k}")
                nc.gpsimd.dma_start(t_[:], xst[k * 128:(k + 1) * 128, :])
                xt.append(t_)

            # ---- W_eff^T moving tiles via strided loads on SP queue ----
            # cc_out rows: r*1024 + k*128 + d', cols: o_local
            cc_view = cc_out[:].rearrange("(r k p) f -> k p r f",
                                          r=N_CORES, k=DB)
            wm = []
            for k in range(DB):
                t_ = wmov.tile([128, D], BF16, tag=f"wm{k}")
                dst = t_[:].rearrange("p (r f) -> p r f", r=N_CORES)
                nc.sync.dma_start(dst, cc_view[k])
                wm.append(t_)

            # ---- main matmuls + LayerNorm per token block ----
            for t in range(TB):
                ph0 = psum_y.tile([128, 512], F32, tag="py")
                ph1 = psum_y.tile([128, 512], F32, tag="py")
                ph = [ph0, ph1]
                for k in range(DB):
                    lhsT = xt[k][:, t * 128:(t + 1) * 128]
                    for h in range(2):
                        nc.tensor.matmul(ph[h][:], lhsT,
                                         wm[k][:, h * 512:(h + 1) * 512],
                                         start=(k == 0), stop=(k == DB - 1))

                # LN stats on DVE: bn_stats per half -> bn_aggr
                bs = stats.tile([128, 12], F32, tag="bs")
                for h in range(2):
                    nc.vector.bn_stats(bs[:, h * 6:(h + 1) * 6], ph[h][:])
                mv = stats.tile([128, 8], F32, tag="mv")
                nc.vector.bn_aggr(mv[:, 0:2], bs[:])
                # a_pre = m^2*var + eps; rs = 1/sqrt(a_pre)
                m = maskc[:, t:t + 1]
                nc.vector.tensor_scalar(
                    mv[:, 2:3], m, m, None, mybir.AluOpType.mult)
                nc.vector.tensor_scalar(
                    mv[:, 3:4], mv[:, 1:2], mv[:, 2:3], LN_EPS,
                    mybir.AluOpType.mult, mybir.AluOpType.add)
                nc.scalar.sqrt(mv[:, 4:5], mv[:, 3:4])
                nc.vector.reciprocal(mv[:, 5:6], mv[:, 4:5])
                # a = m*rs ; b = -mean*a
                nc.vector.tensor_scalar(
                    mv[:, 6:7], mv[:, 5:6], m, None, mybir.AluOpType.mult)
                nc.vector.tensor_scalar(
                    mv[:, 7:8], mv[:, 6:7], mv[:, 0:1], -1.0,
                    mybir.AluOpType.mult, mybir.AluOpType.mult)

                # normalize from PSUM on ACT: z = y*a + b
                if affine:
                    zsb = lnp.tile([128, D], BF16, tag="zsb")
                    for h in range(2):
                        nc.scalar.activation(
                            zsb[:, h * 512:(h + 1) * 512], ph[h][:],
                            mybir.ActivationFunctionType.Identity,
                            bias=mv[:, 7:8], scale=mv[:, 6:7])
                    zg = lnp.tile([128, D], BF16, tag="zg")
                    nc.vector.tensor_tensor(zg[:], zsb[:], gammab[:],
                                            mybir.AluOpType.mult)
                    zf = lnp.tile([128, D], F32, tag="zf")
                    nc.vector.tensor_tensor(zf[:], zg[:], betab[:],
                                            mybir.AluOpType.add)
                else:
                    zf = lnp.tile([128, D], F32, tag="zf")
                    for h in range(2):
                        nc.scalar.activation(
                            zf[:, h * 512:(h + 1) * 512], ph[h][:],
                            mybir.ActivationFunctionType.Identity,
                            bias=mv[:, 7:8], scale=mv[:, 6:7])
                nc.gpsimd.dma_start(out[t * 128:(t + 1) * 128, :], zf[:])

    nc.compile()
    return nc


def _get_nc(affine: bool):
    key = f"nc_{affine}"
    if key not in _cached:
        _cached[key] = _build(affine)
    return _cached[key]


def _host_prep(x, mask, rank_weights, Ws, ln_gamma, ln_beta):
    rw = rank_weights.astype(np.float64)
    e = np.exp(rw - rw.max())
    probs64 = e / e.sum()
    ranks = np.arange(MIN_RANK, MAX_RANK + 1, dtype=np.float64)
    expected_rank = np.float32((probs64 * ranks).sum())
    rank_entropy = np.float32(-(probs64 * np.log(probs64 + 1e-8)).sum())
    probs = probs64.astype(np.float32)

    affine = not (np.all(ln_gamma == 1.0) and np.all(ln_beta == 0.0))

    x2 = np.ascontiguousarray(x.reshape(B * S, D))
    mask2 = mask.reshape(B * S)
    diag = np.zeros((128, R * 128), dtype=np.float32)
    for r in range(R):
        diag[np.arange(128), r * 128 + np.arange(128)] = probs[r]
    diag = diag.astype(NP_BF16)
    ident = np.eye(128, dtype=NP_BF16)
    if affine:
        gamma_b = np.ascontiguousarray(
            np.broadcast_to(ln_gamma[None, :], (128, D))).astype(NP_BF16)
        beta_b = np.ascontiguousarray(
            np.broadcast_to(ln_beta[None, :], (128, D))).astype(NP_BF16)

    in_maps = []
    for i in range(N_CORES):
        m = {
            "xst": np.ascontiguousarray(
                x2[i * T:(i + 1) * T].T).astype(NP_BF16),
            "ws": np.ascontiguousarray(
                Ws[:, i * OL:(i + 1) * OL, :]).astype(NP_BF16),
            "diag": diag,
            "ident": ident,
            "maskc": np.ascontiguousarray(
                mask2[i * T:(i + 1) * T].reshape(TB, 128).T
            ).astype(np.float32),
        }
        if affine:
            m["gammab"] = gamma_b
            m["betab"] = beta_b
        in_maps.append(m)
    return in_maps, affine, expected_rank, rank_entropy


def kernel(x, mask, rank_weights, Ws, ln_gamma, ln_beta):
    in_maps, affine, expected_rank, rank_entropy = _host_prep(
        x, mask, rank_weights, Ws, ln_gamma, ln_beta)
    nc = _get_nc(affine)
    _cached["in_maps"] = in_maps
    _cached["affine"] = affine
    res = bass_utils.run_bass_kernel_spmd(
        nc, in_maps, core_ids=list(range(N_CORES)))
    out = np.concatenate([res.results[i]["out"] for i in range(N_CORES)],
                         axis=0).reshape(B, S, D).astype(np.float32)
    return out, expected_rank, rank_entropy


# revision 30
# speedup vs baseline: 1.1804x; 1.0562x over previous
"""Distributed Trainium2 kernel for AdaptiveLowRank (softmax-weighted sum of
16 linear maps + LayerNorm), SPMD across 8 NeuronCores.

Strategy: data-parallel over tokens; the 16 weight matrices are sharded over
both r and output-dim so each core reads only 1/8 of Ws. The o-slice of
W_eff = sum_r softmax(rank_weights)_r * W_r is combined on the TensorEngine
(p_r*I diagonal matmuls accumulating in PSUM f32), transposed on the PE,
and AllGathered in two d-chunks so the gather overlaps the start of the
main matmul. LayerNorm stats run on DVE (bn_stats); the normalize runs on
ACT straight out of PSUM with the mask folded into the per-token affine.

DMA engine assignment (each engine queue is in-order, so streams that must
not block each other live on different engines):
  SP (sync):   ws loads, strided W_eff^T moving-tile loads
  ACT (scalar): consts, x^T loads, cc_in bounce writes
  GPSIMD:      collectives, output stores
"""

import numpy as np
import ml_dtypes

import concourse.bass as bass
import concourse.mybir as mybir
import concourse.tile as tile
from concourse import bacc
from concourse import bass_utils

N_CORES = 8
B, S, D = 4, 2048, 1024
R = 16
T = (B * S) // N_CORES          # tokens per core
OL = D // N_CORES               # output rows owned per core
TB = T // 128                   # token blocks per core
DB = D // 128                   # contraction blocks
DH = D // 2                     # d-half size (AG chunk)
LN_EPS = 1e-5
MIN_RANK, MAX_RANK = 1, 16

BF16 = mybir.dt.bfloat16
F32 = mybir.dt.float32
NP_BF16 = ml_dtypes.bfloat16

_cached = {}


def _build(affine: bool):
    nc = bacc.Bacc("TRN2", target_bir_lowering=False, debug=False,
                   num_devices=N_CORES)

    xst = nc.dram_tensor("xst", [D, T], BF16, kind="ExternalInput")
    ws = nc.dram_tensor("ws", [R, OL, D], BF16, kind="ExternalInput")
    diag_in = nc.dram_tensor("diag", [128, R * 128], BF16,
                             kind="ExternalInput")
    maskc_in = nc.dram_tensor("maskc", [128, TB], F32, kind="ExternalInput")
    if affine:
        gamma_in = nc.dram_tensor("gammab", [128, D], BF16,
                                  kind="ExternalInput")
        beta_in = nc.dram_tensor("betab", [128, D], BF16,
                                 kind="ExternalInput")
    out = nc.dram_tensor("out", [T, D], F32, kind="ExternalOutput")

    with tile.TileContext(nc, trace_sim=False) as tc:
        with (
            tc.tile_pool(name="dram", bufs=1, space="DRAM") as dram,
            tc.tile_pool(name="consts", bufs=1) as consts,
            tc.tile_pool(name="wld", bufs=16) as wld,
            tc.tile_pool(name="accsb", bufs=2) as accsb,
            tc.tile_pool(name="xt", bufs=1) as xtp,
            tc.tile_pool(name="wmov", bufs=1) as wmov,
            tc.tile_pool(name="psum_w", bufs=2, space="PSUM") as psum_w,
            tc.tile_pool(name="psum_y", bufs=5, space="PSUM") as psum_y,
            tc.tile_pool(name="ln", bufs=3) as lnp,
            tc.tile_pool(name="stats", bufs=3) as stats,
        ):
            # consts on ACT queue (tiny, first)
            diag = consts.tile([128, R * 128], BF16, name="diag")
            nc.scalar.dma_start(diag[:], diag_in[:])
            maskc = consts.tile([128, TB], F32, name="maskc")
            nc.scalar.dma_start(maskc[:], maskc_in[:])
            if affine:
                gammab = consts.tile([128, D], BF16, name="gammab")
                nc.scalar.dma_start(gammab[:], gamma_in[:])
                betab = consts.tile([128, D], BF16, name="betab")
                nc.scalar.dma_start(betab[:], beta_in[:])

            # ACT table pre-warm (Identity + Sqrt)
            warm = stats.tile([128, 2], F32, tag="warm")
            nc.scalar.activation(warm[:, 0:1], maskc[:, 0:1],
                                 mybir.ActivationFunctionType.Identity,
                                 bias=0.0, scale=1.0)
            nc.scalar.sqrt(warm[:, 1:2], maskc[:, 0:1])

            # ws loads on SP queue (pipelined into the PE combine)
            wtiles = {}
            for h in range(2):
                for r in range(R):
                    w_r = wld.tile([OL, DH], BF16, tag="wld")
                    nc.sync.dma_start(
                        w_r[:], ws[r][:, h * DH:(h + 1) * DH])
                    wtiles[(h, r)] = w_r

            # x^T tiles (host pre-transposed) on ACT queue, early
            xt = []
            for k in range(DB):
                t_ = xtp.tile([128, T], BF16, tag=f"xt{k}")
                nc.scalar.dma_start(t_[:], xst[k * 128:(k + 1) * 128, :])
                xt.append(t_)

            # HAM warm-up: junk matmuls on the diag tile, no downstream
            # readers; they run as soon as diag lands and bring the PE to
            # full clock before the real combine matmuls arrive
            jp = psum_w.tile([128, 128], F32, tag="jp", bufs=1)
            for _ in range(36):
                nc.tensor.matmul(jp[:], diag[:, 0:128], diag[:, 0:128],
                                 start=True, stop=True)

            # ---- W-combine on PE -> one AllGather of [o, d] slices ----
            cc_in = dram.tile([OL, D], BF16, name="cc_in")
            for h in range(2):
                pw = psum_w.tile([128, DH], F32, tag="pw")
                for r in range(R):
                    nc.tensor.matmul(pw[:], diag[:, r * 128:(r + 1) * 128],
                                     wtiles[(h, r)][:],
                                     start=(r == 0), stop=(r == R - 1))
                acc_sb = accsb.tile([128, DH], BF16, tag="accsb")
                nc.scalar.copy(acc_sb[:], pw[:])
                nc.scalar.dma_start(cc_in[:, h * DH:(h + 1) * DH], acc_sb[:])
            cc_out = dram.tile([N_CORES * OL, D], BF16,
                               name="cc_out", addr_space="Shared")
            nc.gpsimd.collective_compute(
                "AllGather",
                mybir.AluOpType.bypass,
                replica_groups=[list(range(N_CORES))],
                ins=[cc_in[:].opt()],
                outs=[cc_out[:].opt()],
            )

            # W_eff^T moving tiles via xbar DMA-transpose of W_eff columns
            wm = []
            for k in range(DB):
                t_ = wmov.tile([128, D], BF16, tag=f"wm{k}")
                nc.sync.dma_start(t_[:], cc_out[:, k * 128:(k + 1) * 128],
                                  transpose=True)
                wm.append(t_)

            # ---- main matmuls + LayerNorm per token block ----
            for t in range(TB):
                ph0 = psum_y.tile([128, 512], F32, tag="py")
                ph1 = psum_y.tile([128, 512], F32, tag="py")
                ph = [ph0, ph1]
                for k in range(DB):
                    lhsT = xt[k][:, t * 128:(t + 1) * 128]
                    for h in range(2):
                        nc.tensor.matmul(ph[h][:], lhsT,
                                         wm[k][:, h * 512:(h + 1) * 512],
                                         start=(k == 0), stop=(k == DB - 1))

                # LN stats on DVE: bn_stats per half -> bn_aggr
                bs = stats.tile([128, 12], F32, tag="bs")
                for h in range(2):
                    nc.vector.bn_stats(bs[:, h * 6:(h + 1) * 6], ph[h][:])
                mv = stats.tile([128, 8], F32, tag="mv")
                nc.vector.bn_aggr(mv[:, 0:2], bs[:])
                # a_pre = m^2*var + eps; rs = 1/sqrt(a_pre)
                m = maskc[:, t:t + 1]
                nc.vector.tensor_scalar(
                    mv[:, 2:3], m, m, None, mybir.AluOpType.mult)
                nc.vector.tensor_scalar(
                    mv[:, 3:4], mv[:, 1:2], mv[:, 2:3], LN_EPS,
                    mybir.AluOpType.mult, mybir.AluOpType.add)
                nc.scalar.sqrt(mv[:, 4:5], mv[:, 3:4])
                nc.vector.reciprocal(mv[:, 5:6], mv[:, 4:5])
                # a = m*rs ; b = -mean*a
                nc.vector.tensor_scalar(
                    mv[:, 6:7], mv[:, 5:6], m, None, mybir.AluOpType.mult)
                nc.vector.tensor_scalar(
                    mv[:, 7:8], mv[:, 6:7], mv[:, 0:1], -1.0,
                    mybir.AluOpType.mult, mybir.AluOpType.mult)

                # normalize from PSUM on ACT: z = y*a + b
                if affine:
                    zsb = lnp.tile([128, D], BF16, tag="zsb")
                    for h in range(2):
                        nc.scalar.activation(
                            zsb[:, h * 512:(h + 1) * 512], ph[h][:],
                            mybir.ActivationFunctionType.Identity,
                            bias=mv[:, 7:8], scale=mv[:, 6:7])
                    zg = lnp.tile([128, D], BF16, tag="zg")
                    nc.vector.tensor_tensor(zg[:], zsb[:], gammab[:],
                                            mybir.AluOpType.mult)
                    zf = lnp.tile([128, D], F32, tag="zf")
                    nc.vector.tensor_tensor(zf[:], zg[:], betab[:],
                                            mybir.AluOpType.add)
                else:
                    zf = lnp.tile([128, D], F32, tag="zf")
                    for h in range(2):
                        nc.scalar.activation(
                            zf[:, h * 512:(h + 1) * 512], ph[h][:],
                            mybir.ActivationFunctionType.Identity,
                            bias=mv[:, 7:8], scale=mv[:, 6:7])
                nc.gpsimd.dma_start(out[t * 128:(t + 1) * 128, :], zf[:])

    nc.compile()
    return nc


def _get_nc(affine: bool):
    key = f"nc_{affine}"
    if key not in _cached:
        _cached[key] = _build(affine)
    return _cached[key]


def _host_prep(x, mask, rank_weights, Ws, ln_gamma, ln_beta):
    rw = rank_weights.astype(np.float64)
    e = np.exp(rw - rw.max())
    probs64 = e / e.sum()
    ranks = np.arange(MIN_RANK, MAX_RANK + 1, dtype=np.float64)
    expected_rank = np.float32((probs64 * ranks).sum())
    rank_entropy = np.float32(-(probs64 * np.log(probs64 + 1e-8)).sum())
    probs = probs64.astype(np.float32)

    affine = not (np.all(ln_gamma == 1.0) and np.all(ln_beta == 0.0))

    x2 = np.ascontiguousarray(x.reshape(B * S, D))
    mask2 = mask.reshape(B * S)
    diag = np.zeros((128, R * 128), dtype=np.float32)
    for r in range(R):
        diag[np.arange(128), r * 128 + np.arange(128)] = probs[r]
    diag = diag.astype(NP_BF16)
    if affine:
        gamma_b = np.ascontiguousarray(
            np.broadcast_to(ln_gamma[None, :], (128, D))).astype(NP_BF16)
        beta_b = np.ascontiguousarray(
            np.broadcast_to(ln_beta[None, :], (128, D))).astype(NP_BF16)

    in_maps = []
    for i in range(N_CORES):
        m = {
            "xst": np.ascontiguousarray(
                x2[i * T:(i + 1) * T].T).astype(NP_BF16),
            "ws": np.ascontiguousarray(
                Ws[:, i * OL:(i + 1) * OL, :]).astype(NP_BF16),
            "diag": diag,
            "maskc": np.ascontiguousarray(
                mask2[i * T:(i + 1) * T].reshape(TB, 128).T
            ).astype(np.float32),
        }
        if affine:
            m["gammab"] = gamma_b
            m["betab"] = beta_b
        in_maps.append(m)
    return in_maps, affine, expected_rank, rank_entropy


def kernel(x, mask, rank_weights, Ws, ln_gamma, ln_beta):
    in_maps, affine, expected_rank, rank_entropy = _host_prep(
        x, mask, rank_weights, Ws, ln_gamma, ln_beta)
    nc = _get_nc(affine)
    _cached["in_maps"] = in_maps
    _cached["affine"] = affine
    res = bass_utils.run_bass_kernel_spmd(
        nc, in_maps, core_ids=list(range(N_CORES)))
    out = np.concatenate([res.results[i]["out"] for i in range(N_CORES)],
                         axis=0).reshape(B, S, D).astype(np.float32)
    return out, expected_rank, rank_entropy


# revision 36
# speedup vs baseline: 1.2593x; 1.0669x over previous
"""Distributed Trainium2 kernel for AdaptiveLowRank (softmax-weighted sum of
16 linear maps + LayerNorm), SPMD across 8 NeuronCores.

Strategy: data-parallel over tokens; the 16 weight matrices are sharded over
both r and output-dim so each core reads only 1/8 of Ws. The o-slice of
W_eff = sum_r softmax(rank_weights)_r * W_r is combined on the TensorEngine
(p_r*I diagonal matmuls accumulating in PSUM f32), transposed on the PE,
and AllGathered in two d-chunks so the gather overlaps the start of the
main matmul. LayerNorm stats run on DVE (bn_stats); the normalize runs on
ACT straight out of PSUM with the mask folded into the per-token affine.

DMA engine assignment (each engine queue is in-order, so streams that must
not block each other live on different engines):
  SP (sync):   ws loads, strided W_eff^T moving-tile loads
  ACT (scalar): consts, x^T loads, cc_in bounce writes
  GPSIMD:      collectives, output stores
"""

import numpy as np
import ml_dtypes

import concourse.bass as bass
import concourse.mybir as mybir
import concourse.tile as tile
from concourse import bacc
from concourse import bass_utils

N_CORES = 8
B, S, D = 4, 2048, 1024
R = 16
T = (B * S) // N_CORES          # tokens per core
OL = D // N_CORES               # output rows owned per core
TB = T // 128                   # token blocks per core
DB = D // 128                   # contraction blocks
DH = D // 2                     # d-half size (AG chunk)
LN_EPS = 1e-5
MIN_RANK, MAX_RANK = 1, 16

BF16 = mybir.dt.bfloat16
F32 = mybir.dt.float32
NP_BF16 = ml_dtypes.bfloat16

_cached = {}


def _build(affine: bool):
    nc = bacc.Bacc("TRN2", target_bir_lowering=False, debug=False,
                   num_devices=N_CORES)

    xst = nc.dram_tensor("xst", [D, T], BF16, kind="ExternalInput")
    ws = nc.dram_tensor("ws", [R, OL, D], BF16, kind="ExternalInput")
    diag_in = nc.dram_tensor("diag", [128, R * 128], BF16,
                             kind="ExternalInput")
    maskc_in = nc.dram_tensor("maskc", [128, TB], F32, kind="ExternalInput")
    if affine:
        gamma_in = nc.dram_tensor("gammab", [128, D], BF16,
                                  kind="ExternalInput")
        beta_in = nc.dram_tensor("betab", [128, D], BF16,
                                 kind="ExternalInput")
    out = nc.dram_tensor("out", [T, D], F32, kind="ExternalOutput")

    with tile.TileContext(nc, trace_sim=False) as tc:
        with (
            tc.tile_pool(name="dram", bufs=1, space="DRAM") as dram,
            tc.tile_pool(name="consts", bufs=1) as consts,
            tc.tile_pool(name="wld", bufs=16) as wld,
            tc.tile_pool(name="accsb", bufs=2) as accsb,
            tc.tile_pool(name="xt", bufs=1) as xtp,
            tc.tile_pool(name="wmov", bufs=1) as wmov,
            tc.tile_pool(name="psum_w", bufs=2, space="PSUM") as psum_w,
            tc.tile_pool(name="psum_y", bufs=5, space="PSUM") as psum_y,
            tc.tile_pool(name="ln", bufs=3) as lnp,
            tc.tile_pool(name="stats", bufs=3) as stats,
        ):
            # consts on ACT queue (tiny, first)
            diag = consts.tile([128, R * 128], BF16, name="diag")
            nc.scalar.dma_start(diag[:], diag_in[:])
            maskc = consts.tile([128, TB], F32, name="maskc")
            nc.scalar.dma_start(maskc[:], maskc_in[:])
            if affine:
                gammab = consts.tile([128, D], BF16, name="gammab")
                nc.scalar.dma_start(gammab[:], gamma_in[:])
                betab = consts.tile([128, D], BF16, name="betab")
                nc.scalar.dma_start(betab[:], beta_in[:])

            # ACT table pre-warm (Identity + Sqrt)
            warm = stats.tile([128, 2], F32, tag="warm")
            nc.scalar.activation(warm[:, 0:1], maskc[:, 0:1],
                                 mybir.ActivationFunctionType.Identity,
                                 bias=0.0, scale=1.0)
            nc.scalar.sqrt(warm[:, 1:2], maskc[:, 0:1])

            # ws loads on SP queue: full rows (contiguous 2KB bursts),
            # pipelined into the PE combine per r
            wtiles = []
            for r in range(R):
                w_r = wld.tile([OL, D], BF16, tag="wld")
                nc.sync.dma_start(w_r[:], ws[r])
                wtiles.append(w_r)

            # x^T tiles (host pre-transposed) on ACT queue, early
            xt = []
            for k in range(DB):
                t_ = xtp.tile([128, T], BF16, tag=f"xt{k}")
                nc.scalar.dma_start(t_[:], xst[k * 128:(k + 1) * 128, :])
                xt.append(t_)

            # HAM warm-up: junk matmuls on the diag tile, no downstream
            # readers; they run as soon as diag lands and bring the PE to
            # full clock before the real combine matmuls arrive
            jp = psum_w.tile([128, 128], F32, tag="jp", bufs=1)
            for _ in range(36):
                nc.tensor.matmul(jp[:], diag[:, 0:128], diag[:, 0:128],
                                 start=True, stop=True)

            # ---- W-combine on PE -> one AllGather of [o, d] slices ----
            cc_in = dram.tile([OL, D], BF16, name="cc_in")
            for h in range(2):
                pw = psum_w.tile([128, DH], F32, tag="pw")
                for r in range(R):
                    nc.tensor.matmul(pw[:], diag[:, r * 128:(r + 1) * 128],
                                     wtiles[r][:, h * DH:(h + 1) * DH],
                                     start=(r == 0), stop=(r == R - 1))
                acc_sb = accsb.tile([128, DH], BF16, tag="accsb")
                if h == 0:
                    nc.scalar.copy(acc_sb[:], pw[:])
                else:
                    nc.vector.tensor_copy(acc_sb[:], pw[:])
                nc.scalar.dma_start(cc_in[:, h * DH:(h + 1) * DH], acc_sb[:])
            cc_out = dram.tile([N_CORES * OL, D], BF16,
                               name="cc_out", addr_space="Shared")
            nc.gpsimd.collective_compute(
                "AllGather",
                mybir.AluOpType.bypass,
                replica_groups=[list(range(N_CORES))],
                ins=[cc_in[:].opt()],
                outs=[cc_out[:].opt()],
            )

            # W_eff^T moving tiles via xbar DMA-transpose of W_eff columns
            # (single engine: concurrent xbar streams corrupt data)
            wm = []
            for k in range(DB):
                t_ = wmov.tile([128, D], BF16, tag=f"wm{k}")
                nc.sync.dma_start(t_[:], cc_out[:, k * 128:(k + 1) * 128],
                                  transpose=True)
                wm.append(t_)

            # ---- main matmuls + LayerNorm per token block ----
            for t in range(TB):
                ph0 = psum_y.tile([128, 512], F32, tag="py")
                ph1 = psum_y.tile([128, 512], F32, tag="py")
                ph = [ph0, ph1]
                for k in range(DB):
                    lhsT = xt[k][:, t * 128:(t + 1) * 128]
                    for h in range(2):
                        nc.tensor.matmul(ph[h][:], lhsT,
                                         wm[k][:, h * 512:(h + 1) * 512],
                                         start=(k == 0), stop=(k == DB - 1))

                # LN stats on DVE: bn_stats per half -> bn_aggr
                bs = stats.tile([128, 12], F32, tag="bs")
                for h in range(2):
                    nc.vector.bn_stats(bs[:, h * 6:(h + 1) * 6], ph[h][:])
                mv = stats.tile([128, 8], F32, tag="mv")
                nc.vector.bn_aggr(mv[:, 0:2], bs[:])
                # a_pre = m^2*var + eps; rs = 1/sqrt(a_pre)
                m = maskc[:, t:t + 1]
                nc.vector.tensor_scalar(
                    mv[:, 2:3], m, m, None, mybir.AluOpType.mult)
                nc.vector.tensor_scalar(
                    mv[:, 3:4], mv[:, 1:2], mv[:, 2:3], LN_EPS,
                    mybir.AluOpType.mult, mybir.AluOpType.add)
                nc.scalar.sqrt(mv[:, 4:5], mv[:, 3:4])
                nc.vector.reciprocal(mv[:, 5:6], mv[:, 4:5])
                # a = m*rs ; b = -mean*a
                nc.vector.tensor_scalar(
                    mv[:, 6:7], mv[:, 5:6], m, None, mybir.AluOpType.mult)
                nc.vector.tensor_scalar(
                    mv[:, 7:8], mv[:, 6:7], mv[:, 0:1], -1.0,
                    mybir.AluOpType.mult, mybir.AluOpType.mult)

                # normalize from PSUM on ACT: z = y*a + b
                if affine:
                    zsb = lnp.tile([128, D], BF16, tag="zsb")
                    for h in range(2):
                        nc.scalar.activation(
                            zsb[:, h * 512:(h + 1) * 512], ph[h][:],
                            mybir.ActivationFunctionType.Identity,
                            bias=mv[:, 7:8], scale=mv[:, 6:7])
                    zg = lnp.tile([128, D], BF16, tag="zg")
                    nc.vector.tensor_tensor(zg[:], zsb[:], gammab[:],
                                            mybir.AluOpType.mult)
                    zf = lnp.tile([128, D], F32, tag="zf")
                    nc.vector.tensor_tensor(zf[:], zg[:], betab[:],
                                            mybir.AluOpType.add)
                else:
                    zf = lnp.tile([128, D], F32, tag="zf")
                    for h in range(2):
                        nc.scalar.activation(
                            zf[:, h * 512:(h + 1) * 512], ph[h][:],
                            mybir.ActivationFunctionType.Identity,
                            bias=mv[:, 7:8], scale=mv[:, 6:7])
                nc.gpsimd.dma_start(out[t * 128:(t + 1) * 128, :], zf[:])

    nc.compile()
    return nc


def _get_nc(affine: bool):
    key = f"nc_{affine}"
    if key not in _cached:
        _cached[key] = _build(affine)
    return _cached[key]


def _host_prep(x, mask, rank_weights, Ws, ln_gamma, ln_beta):
    rw = rank_weights.astype(np.float64)
    e = np.exp(rw - rw.max())
    probs64 = e / e.sum()
    ranks = np.arange(MIN_RANK, MAX_RANK + 1, dtype=np.float64)
    expected_rank = np.float32((probs64 * ranks).sum())
    rank_entropy = np.float32(-(probs64 * np.log(probs64 + 1e-8)).sum())
    probs = probs64.astype(np.float32)

    affine = not (np.all(ln_gamma == 1.0) and np.all(ln_beta == 0.0))

    x2 = np.ascontiguousarray(x.reshape(B * S, D))
    mask2 = mask.reshape(B * S)
    diag = np.zeros((128, R * 128), dtype=np.float32)
    for r in range(R):
        diag[np.arange(128), r * 128 + np.arange(128)] = probs[r]
    diag = diag.astype(NP_BF16)
    if affine:
        gamma_b = np.ascontiguousarray(
            np.broadcast_to(ln_gamma[None, :], (128, D))).astype(NP_BF16)
        beta_b = np.ascontiguousarray(
            np.broadcast_to(ln_beta[None, :], (128, D))).astype(NP_BF16)

    in_maps = []
    for i in range(N_CORES):
        m = {
            "xst": np.ascontiguousarray(
                x2[i * T:(i + 1) * T].T).astype(NP_BF16),
            "ws": np.ascontiguousarray(
                Ws[:, i * OL:(i + 1) * OL, :]).astype(NP_BF16),
            "diag": diag,
            "maskc": np.ascontiguousarray(
                mask2[i * T:(i + 1) * T].reshape(TB, 128).T
            ).astype(np.float32),
        }
        if affine:
            m["gammab"] = gamma_b
            m["betab"] = beta_b
        in_maps.append(m)
    return in_maps, affine, expected_rank, rank_entropy


def kernel(x, mask, rank_weights, Ws, ln_gamma, ln_beta):
    in_maps, affine, expected_rank, rank_entropy = _host_prep(
        x, mask, rank_weights, Ws, ln_gamma, ln_beta)
    nc = _get_nc(affine)
    _cached["in_maps"] = in_maps
    _cached["affine"] = affine
    res = bass_utils.run_bass_kernel_spmd(
        nc, in_maps, core_ids=list(range(N_CORES)))
    out = np.concatenate([res.results[i]["out"] for i in range(N_CORES)],
                         axis=0).reshape(B, S, D).astype(np.float32)
    return out, expected_rank, rank_entropy
